# revision 5
# baseline (speedup 1.0000x reference)
# Binary (sign) matmul: out[b,m,n] = sum_k sign(x[b,m,k]) * sign(y[b,n,k]) * x_clip * y_clip
# B=2, M=N=K=4096, fp32 in/out.
#
# Sharding: 8 cores = batch(2) x 2x2 grid over (M, N). Each core computes a
# [2048, 2048] output block from x[b, mh*2048:, :] and y[b, nh*2048:, :].
# The host binds each core's shards in k-major (transposed) layout — pure
# input marshalling; all arithmetic (sign, matmul, clip scaling) runs on
# device.
#
# Per-core device pipeline:
#   DMA fp32 k-major half-chunks -> ScalarE Sign (fp32 -> fp8e4 +-1) ->
#   TensorE DoubleRow fp8 matmuls (exact: sums of +-1 in fp32 PSUM) ->
#   DVE spills/close -> DMA out.
#
# Schedule: K split in three phases [4,4,8] DR-steps (kd = 256 k-values).
#   q0 (kd 0-3):  per i-tile, accumulate in PSUM, spill CAST -> fp16 ACC
#                 (|sum| <= 1024, exact in fp16)
#   q1 (kd 4-7):  accumulate, spill ADD into ACC (|sum| <= 2048, exact)
#   H  (kd 8-15): accumulate 8 steps in PSUM, close: fp32 add of PSUM+ACC,
#                 scale by clip product, DMA out.
# All 16 i-tiles march through each phase in sequence; phase boundaries
# match the DMA stream order (k-ascending), so the PE chases the stream
# with bounded stalls instead of serializing behind a full-K dependency.
#
# HAM warmth: PE-idle gaps > ~3.4us re-throttle the PE clock to 1.2 GHz.
# During the two chase windows (q0-i0 and H-i0) the PE waits on chunk
# landings; tiny normal-mode fp8 matmuls that READ freshly-signed SxT/SyT
# slices fire as the Sign ops complete (~3us cadence), keeping the
# activity monitor busy through the gaps without fake time dependencies.
import numpy as np

B = 2
M = N = K = 4096
P = 128
MSH, NSH = 2048, 2048      # per-core shard of M, N
KO = K // P                # 32 k-chunks of 128
MT = MSH // P              # 16 m row-tiles
FD = 512                   # matmul free dim
NCH = NSH // FD            # 4 n chunks
NCORES = 8

KD = KO // 2               # 16 DoubleRow k-steps of 256
Q0 = range(0, 4)           # phase kd ranges
Q1 = range(4, 8)
HH = range(8, 16)
HCH = 1024                 # half-chunk columns for DMA/Sign staging


def _build_program():
    import concourse.bacc as bacc
    import concourse.mybir as mybir
    import concourse.tile as tile
    from concourse.bass import ts

    f32 = mybir.dt.float32
    f16 = mybir.dt.float16
    fp8 = mybir.dt.float8e4
    Sign = mybir.ActivationFunctionType.Sign

    nc = bacc.Bacc(
        "TRN2",
        target_bir_lowering=False,
        debug=False,
        num_devices=NCORES,
    )
    xsT = nc.dram_tensor("xsT", [K, MSH], f32, kind="ExternalInput").ap()
    ysT = nc.dram_tensor("ysT", [K, NSH], f32, kind="ExternalInput").ap()
    clips = nc.dram_tensor("clips", [P, 2], f32, kind="ExternalInput").ap()
    out = nc.dram_tensor("out", [MSH, NSH], f32, kind="ExternalOutput").ap()

    with tile.TileContext(nc) as tc:
        with (
            tc.tile_pool(name="constp", bufs=1) as constp,
            tc.tile_pool(name="sytp", bufs=1) as sytp,
            tc.tile_pool(name="sxtp", bufs=1) as sxtp,
            tc.tile_pool(name="accp", bufs=1) as accp,
            tc.tile_pool(name="stagep", bufs=2) as stagep,
            tc.tile_pool(name="outp", bufs=3) as outp,
            tc.tile_pool(name="psump", bufs=7, space="PSUM") as psump,
            tc.tile_pool(name="dpsump", bufs=1, space="PSUM") as dpsump,
        ):
            # clip product, replicated per-partition: [P, 1]
            clip_sb = constp.tile([P, 2], f32)
            nc.sync.dma_start(clip_sb[:], clips)
            clip_prod = constp.tile([P, 1], f32)
            nc.vector.tensor_tensor(
                clip_prod[:], clip_sb[:, 0:1], clip_sb[:, 1:2],
                mybir.AluOpType.mult,
            )

            # SxT[ki, ko, m] = sign(x[m, ko*P + ki]) as fp8; SyT likewise.
            SxT = sxtp.tile([P, KO, MSH], fp8)
            SyT = sytp.tile([P, KO, NSH], fp8)
            # fp16 accumulator for the two quarter-K spills (exact <= 2048)
            ACC = accp.tile([P, MT, NCH, FD], f16, name="ACC")

            # Warm-up matmul source: tiny fp8 zero stationary. Warm matmuls
            # read a 32-col slice of a just-signed chunk so they naturally
            # fire at Sign completion times (every ~3us during the stream).
            dwarm = constp.tile([P, 2], fp8)
            nc.vector.memset(dwarm[:], 0)

            def warm_mm(src, ko):
                dps = dpsump.tile([2, 32], f32, name="dps", tag="dps")
                nc.tensor.matmul(
                    dps[:], lhsT=dwarm[:, :2], rhs=src[:, ko, 0:32],
                    start=True, stop=True,
                )

            def prep(src_dram, ko, half, dst, mx):
                st = stagep.tile([P, HCH], f32, name="st", tag="stage")
                nc.sync.dma_start(
                    st[:], src_dram[ts(ko, P), half * HCH : (half + 1) * HCH]
                )
                nc.scalar.activation(
                    dst[:, ko, half * HCH : (half + 1) * HCH], st[:], Sign
                )

            # Stream k-chunks: x and y interleaved, half-chunks for pipelining.
            for ko in range(KO):
                for h in range(2):
                    prep(xsT, ko, h, SxT, MSH)
                for h in range(2):
                    prep(ysT, ko, h, SyT, NSH)

            def mm_group(i, kds, first_kd, last_kd, chase=False):
                """One i-tile's accumulation over kds into 4 fresh psum banks.

                chase=True: this i-tile's kd steps wait on chunk landings;
                before each kd group, emit warm matmuls that read that kd's
                chunks so they fire at Sign-completion cadence (~3us apart)
                and keep HAM warm through the wait."""
                pss = [
                    psump.tile([P, FD], f32, name=f"ps{n}", tag="ps")
                    for n in range(NCH)
                ]
                for kd in kds:
                    if chase:
                        for ko in (2 * kd, 2 * kd + 1):
                            warm_mm(SxT, ko)
                            warm_mm(SyT, ko)
                    for nch in range(NCH):
                        nc.tensor.matmul(
                            pss[nch][:],
                            lhsT=SxT[:, 2 * kd : 2 * kd + 2, ts(i, P)],
                            rhs=SyT[:, 2 * kd : 2 * kd + 2, ts(nch, FD)],
                            start=(kd == first_kd),
                            stop=(kd == last_kd),
                            perf_mode=mybir.MatmulPerfMode.DoubleRow,
                        )
                return pss

            # ---- q0: kd 0..3, spill CAST -> ACC ----
            for i in range(MT):
                pss = mm_group(i, Q0, 0, 3, chase=(i == 0))
                for nch in range(NCH):
                    nc.vector.tensor_copy(
                        out=ACC[:, i, nch, :], in_=pss[nch][:]
                    )

            # ---- q1: kd 4..7, spill ADD -> ACC ----
            for i in range(MT):
                pss = mm_group(i, Q1, 4, 7)
                for nch in range(NCH):
                    nc.vector.tensor_tensor(
                        ACC[:, i, nch, :], pss[nch][:], ACC[:, i, nch, :],
                        mybir.AluOpType.add,
                    )

            # ---- H: kd 8..15, close: PSUM + ACC, scale, out ----
            for i in range(MT):
                pss = mm_group(i, HH, 8, 15, chase=(i == 0))
                for nch in range(NCH):
                    ot = outp.tile([P, FD], f32, name="ot")
                    nc.vector.tensor_tensor(
                        ot[:], pss[nch][:], ACC[:, i, nch, :],
                        mybir.AluOpType.add,
                    )
                    nc.vector.tensor_scalar_mul(ot[:], ot[:], clip_prod[:])
                    nc.sync.dma_start(out[ts(i, P), ts(nch, FD)], ot[:])

    nc.compile()
    _dedupe_ldweights(nc)
    return nc


def _dedupe_ldweights(nc):
    """Drop redundant standalone InstLdweights left by bacc's matmul split.

    Consecutive matmuls sharing one stationary tile still get one
    InstLdweights each; an InstLdweights identical to the previous one
    (same AP, same mode) with no semaphore waits/updates is a no-op."""
    removed = 0
    for blk in nc.m.functions[0].blocks:
        prev_key = None
        keep = []
        for inst in blk.instructions:
            nm = type(inst).__name__
            if nm == "InstLdweights":
                pap = inst.ins[0]
                key = (
                    pap.memref,
                    pap.offset,
                    str(pap.ap),
                    str(pap.dtype),
                    str(inst.perf_mode),
                    str(inst.is_transpose),
                )
                if (
                    key == prev_key
                    and not inst.has_wait()
                    and not inst.has_update()
                ):
                    removed += 1
                    continue
                prev_key = key
            keep.append(inst)
        if removed:
            blk.instructions = keep
    return removed


_PROGRAM_CACHE = None
_LDW_PATCHED = False


def _patch_ldw_opt():
    """Re-enable walrus's LDWEIGHTS elision (consecutive matmuls sharing a
    stationary tile skip the reload). bass_utils hardcodes it off."""
    global _LDW_PATCHED
    if _LDW_PATCHED:
        return
    import concourse.bass_utils as _bu

    _orig = _bu.run_command

    def _run(argv, **kwargs):
        if isinstance(argv, list):
            argv = [
                "--enable-ldw-opt=true" if a == "--enable-ldw-opt=false" else a
                for a in argv
            ]
        return _orig(argv, **kwargs)

    _bu.run_command = _run
    _LDW_PATCHED = True


def _get_program():
    global _PROGRAM_CACHE
    if _PROGRAM_CACHE is None:
        _PROGRAM_CACHE = _build_program()
    return _PROGRAM_CACHE


def _shard_inputs(x, y, x_clip, y_clip):
    x = np.asarray(x, dtype=np.float32)
    y = np.asarray(y, dtype=np.float32)
    clips = np.empty((P, 2), dtype=np.float32)
    clips[:, 0] = np.float32(x_clip)
    clips[:, 1] = np.float32(y_clip)
    in_maps = []
    for c in range(NCORES):
        b, mh, nh = c // 4, (c % 4) // 2, c % 2
        in_maps.append(
            {
                "xsT": np.ascontiguousarray(x[b, mh * MSH : (mh + 1) * MSH, :].T),
                "ysT": np.ascontiguousarray(y[b, nh * NSH : (nh + 1) * NSH, :].T),
                "clips": clips,
            }
        )
    return in_maps


def run_sharded(x, y, x_clip, y_clip, trace=False, **kwargs):
    """Run the SPMD kernel; returns (out, BassKernelResults)."""
    from concourse.bass_utils import run_bass_kernel_spmd

    nc = _get_program()
    in_maps = _shard_inputs(x, y, x_clip, y_clip)
    res = run_bass_kernel_spmd(
        nc, in_maps, core_ids=list(range(NCORES)), trace=trace, **kwargs
    )
    out = np.empty((B, M, N), dtype=np.float32)
    for c in range(NCORES):
        b, mh, nh = c // 4, (c % 4) // 2, c % 2
        out[b, mh * MSH : (mh + 1) * MSH, nh * NSH : (nh + 1) * NSH] = res.results[
            c
        ]["out"]
    return out, res


def kernel(x, y, x_clip, y_clip):
    out, _ = run_sharded(x, y, x_clip, y_clip, trace=False)
    return out


# revision 9
# speedup vs baseline: 1.6126x; 1.6126x over previous
# Binary (sign) matmul: out[b,m,n] = sum_k sign(x[b,m,k]) * sign(y[b,n,k]) * x_clip * y_clip
# B=2, M=N=K=4096, fp32 in/out.
#
# Sharding: 8 cores = batch(2) x 2x2 grid over (M, N). Each core computes a
# [2048, 2048] output block from x[b, mh*2048:, :] and y[b, nh*2048:, :].
# The host binds each core's shards in k-major (transposed) layout — pure
# input marshalling; all arithmetic (sign, matmul, clip scaling) runs on
# device.
#
# Per-core device pipeline:
#   DMA fp32 k-major chunks -> ScalarE Sign (fp32 -> fp8e4 +-1, one op per
#   chunk so downstream LDWEIGHTS dedupe keeps working) -> TensorE DoubleRow
#   fp8 matmuls (exact: sums of +-1 in fp32 PSUM) -> DVE spills/close ->
#   DMA out.
#
# Schedule: K split in three phases [4,4,8] DR-steps (kd = 256 k-values).
#   q0 (kd 0-3):  per i-tile, accumulate in PSUM, spill CAST -> fp16 ACC
#                 (|sum| <= 1024, exact in fp16)
#   q1 (kd 4-7):  accumulate, spill ADD into ACC (|sum| <= 2048, exact)
#   H  (kd 8-15): accumulate 8 steps in PSUM, close: fp32 add of PSUM+ACC,
#                 scale by clip product, DMA out.
# All 16 i-tiles march through each phase in sequence; phase boundaries
# match the DMA stream order (k-ascending), so the PE chases the stream
# with bounded stalls instead of serializing behind a full-K dependency.
#
# SBUF: the fp16 ACC is 64KB/partition, so the sign tensors cannot be fully
# resident. They are held as per-phase slabs (8 k-chunks = 16KB each) in
# 3-buffer pools: a phase's slab is dead once its i-march completes, and the
# pool rotation lets the H slabs overwrite the q0 slabs mid-stream.
#
# HAM warmth: PE-idle gaps > ~3.4us re-throttle the PE clock to 1.2 GHz.
# During the two chase windows (q0-i0 and H-i0) the PE waits on chunk
# landings; tiny normal-mode fp8 matmuls that READ freshly-signed slab
# slices fire as the Sign ops complete (~3us cadence), keeping the
# activity monitor busy through the gaps.
import numpy as np

B = 2
M = N = K = 4096
P = 128
MSH, NSH = 2048, 2048      # per-core shard of M, N
KO = K // P                # 32 k-chunks of 128
MT = MSH // P              # 16 m row-tiles
FD = 512                   # matmul free dim
NCH = NSH // FD            # 4 n chunks
NCORES = 8

KD = KO // 2               # 16 DoubleRow k-steps of 256
SLAB = 8                   # k-chunks per slab (4 DR steps)
NSLAB = KO // SLAB         # 4 slabs: q0, q1, Ha, Hb


def _build_program():
    import concourse.bacc as bacc
    import concourse.mybir as mybir
    import concourse.tile as tile
    from concourse.bass import ts

    f32 = mybir.dt.float32
    f16 = mybir.dt.float16
    fp8 = mybir.dt.float8e4
    Sign = mybir.ActivationFunctionType.Sign

    nc = bacc.Bacc(
        "TRN2",
        target_bir_lowering=False,
        debug=False,
        num_devices=NCORES,
    )
    xsT = nc.dram_tensor("xsT", [K, MSH], f32, kind="ExternalInput").ap()
    ysT = nc.dram_tensor("ysT", [K, NSH], f32, kind="ExternalInput").ap()
    clips = nc.dram_tensor("clips", [P, 2], f32, kind="ExternalInput").ap()
    out = nc.dram_tensor("out", [MSH, NSH], f32, kind="ExternalOutput").ap()

    with tile.TileContext(nc) as tc:
        with (
            tc.tile_pool(name="constp", bufs=1) as constp,
            tc.tile_pool(name="sxp", bufs=3) as sxp,
            tc.tile_pool(name="syp", bufs=3) as syp,
            tc.tile_pool(name="accp", bufs=1) as accp,
            tc.tile_pool(name="stagep", bufs=5) as stagep,
            tc.tile_pool(name="outp", bufs=3) as outp,
            tc.tile_pool(name="psump", bufs=7, space="PSUM") as psump,
            tc.tile_pool(name="dpsump", bufs=1, space="PSUM") as dpsump,
        ):
            # clip product, replicated per-partition: [P, 1]
            clip_sb = constp.tile([P, 2], f32)
            nc.sync.dma_start(clip_sb[:], clips)
            clip_prod = constp.tile([P, 1], f32)
            nc.vector.tensor_tensor(
                clip_prod[:], clip_sb[:, 0:1], clip_sb[:, 1:2],
                mybir.AluOpType.mult,
            )

            # fp16 accumulator for the two quarter-K spills (exact <= 2048)
            ACC = accp.tile([P, MT, NCH, FD], f16, name="ACC")

            # Warm-up matmul source: tiny fp8 zero stationary.
            dwarm = constp.tile([P, 2], fp8)
            nc.vector.memset(dwarm[:], 0)

            def warm_mm(src, kol):
                dps = dpsump.tile([2, 32], f32, name="dps", tag="dps")
                nc.tensor.matmul(
                    dps[:], lhsT=dwarm[:, :2], rhs=src[:, kol, 0:32],
                    start=True, stop=True,
                )

            def prep(src_dram, ko, dst, kol):
                st = stagep.tile([P, MSH], f32, name="st", tag="stage")
                nc.sync.dma_start(st[:], src_dram[ts(ko, P), :])
                nc.scalar.activation(dst[:, kol, :], st[:], Sign)

            # Sign slabs, filled in stream order (x and y interleaved per ko).
            # Slab s covers k-chunks [s*SLAB, (s+1)*SLAB). With 3-buffer
            # pools, slab 3 reuses slab 0's memory; its preps are emitted
            # after the q0 march (the last reader of slab 0) so the pool's
            # WAR tracking sees the reads first in program order.
            xslabs, yslabs = [], []

            def emit_slab(s):
                sx = sxp.tile([P, SLAB, MSH], fp8, name=f"sx{s}", tag="sx")
                sy = syp.tile([P, SLAB, NSH], fp8, name=f"sy{s}", tag="sy")
                xslabs.append(sx)
                yslabs.append(sy)
                for kol in range(SLAB):
                    ko = s * SLAB + kol
                    prep(xsT, ko, sx, kol)
                    prep(ysT, ko, sy, kol)

            for s in range(3):
                emit_slab(s)

            def mm_group(i, sxs, sys, kds, chase):
                """One i-tile's accumulation over DR k-steps into 4 psum banks.

                kds: list of (slab_idx, local_kd). chase: emit warm matmuls
                before each kd group so they fire at Sign cadence during
                chunk-landing waits."""
                pss = [
                    psump.tile([P, FD], f32, name=f"ps{n}", tag="ps")
                    for n in range(NCH)
                ]
                last = len(kds) - 1
                for j, (s, kdl) in enumerate(kds):
                    if chase:
                        for kol in (2 * kdl, 2 * kdl + 1):
                            warm_mm(xslabs[s], kol)
                            warm_mm(yslabs[s], kol)
                    for nch in range(NCH):
                        nc.tensor.matmul(
                            pss[nch][:],
                            lhsT=sxs[s][:, 2 * kdl : 2 * kdl + 2, ts(i, P)],
                            rhs=sys[s][:, 2 * kdl : 2 * kdl + 2, ts(nch, FD)],
                            start=(j == 0),
                            stop=(j == last),
                            perf_mode=mybir.MatmulPerfMode.DoubleRow,
                        )
                return pss

            Q0 = [(0, kdl) for kdl in range(4)]
            Q1 = [(1, kdl) for kdl in range(4)]
            HH = [(2, kdl) for kdl in range(4)] + [(3, kdl) for kdl in range(4)]

            # ---- q0: kd 0..3, spill CAST -> ACC ----
            for i in range(MT):
                pss = mm_group(i, xslabs, yslabs, Q0, chase=(i == 0))
                for nch in range(NCH):
                    nc.vector.tensor_copy(
                        out=ACC[:, i, nch, :], in_=pss[nch][:]
                    )

            # slab 3 (ko 24-31) reuses slab 0's buffers — emit after q0.
            emit_slab(3)

            # ---- q1: kd 4..7, spill ADD -> ACC ----
            for i in range(MT):
                pss = mm_group(i, xslabs, yslabs, Q1, chase=False)
                for nch in range(NCH):
                    nc.vector.tensor_tensor(
                        ACC[:, i, nch, :], pss[nch][:], ACC[:, i, nch, :],
                        mybir.AluOpType.add,
                    )

            # ---- H: kd 8..15, close: PSUM + ACC, scale, out ----
            for i in range(MT):
                pss = mm_group(i, xslabs, yslabs, HH, chase=(i == 0))
                for nch in range(NCH):
                    ot = outp.tile([P, FD], f32, name="ot")
                    nc.vector.tensor_tensor(
                        ot[:], pss[nch][:], ACC[:, i, nch, :],
                        mybir.AluOpType.add,
                    )
                    nc.vector.tensor_scalar_mul(ot[:], ot[:], clip_prod[:])
                    nc.sync.dma_start(out[ts(i, P), ts(nch, FD)], ot[:])

    nc.compile()
    _dedupe_ldweights(nc)
    return nc


def _dedupe_ldweights(nc):
    """Drop redundant standalone InstLdweights left by bacc's matmul split.

    Consecutive matmuls sharing one stationary tile still get one
    InstLdweights each; an InstLdweights identical to the previous one
    (same AP, same mode) with no semaphore waits/updates is a no-op."""
    removed = 0
    for blk in nc.m.functions[0].blocks:
        prev_key = None
        keep = []
        for inst in blk.instructions:
            nm = type(inst).__name__
            if nm == "InstLdweights":
                pap = inst.ins[0]
                key = (
                    pap.memref,
                    pap.offset,
                    str(pap.ap),
                    str(pap.dtype),
                    str(inst.perf_mode),
                    str(inst.is_transpose),
                )
                if (
                    key == prev_key
                    and not inst.has_wait()
                    and not inst.has_update()
                ):
                    removed += 1
                    continue
                prev_key = key
            keep.append(inst)
        if removed:
            blk.instructions = keep
    return removed


_PROGRAM_CACHE = None


def _get_program():
    global _PROGRAM_CACHE
    if _PROGRAM_CACHE is None:
        _PROGRAM_CACHE = _build_program()
    return _PROGRAM_CACHE


def _shard_inputs(x, y, x_clip, y_clip):
    x = np.asarray(x, dtype=np.float32)
    y = np.asarray(y, dtype=np.float32)
    clips = np.empty((P, 2), dtype=np.float32)
    clips[:, 0] = np.float32(x_clip)
    clips[:, 1] = np.float32(y_clip)
    in_maps = []
    for c in range(NCORES):
        b, mh, nh = c // 4, (c % 4) // 2, c % 2
        in_maps.append(
            {
                "xsT": np.ascontiguousarray(x[b, mh * MSH : (mh + 1) * MSH, :].T),
                "ysT": np.ascontiguousarray(y[b, nh * NSH : (nh + 1) * NSH, :].T),
                "clips": clips,
            }
        )
    return in_maps


def run_sharded(x, y, x_clip, y_clip, trace=False, **kwargs):
    """Run the SPMD kernel; returns (out, BassKernelResults)."""
    from concourse.bass_utils import run_bass_kernel_spmd

    nc = _get_program()
    in_maps = _shard_inputs(x, y, x_clip, y_clip)
    res = run_bass_kernel_spmd(
        nc, in_maps, core_ids=list(range(NCORES)), trace=trace, **kwargs
    )
    out = np.empty((B, M, N), dtype=np.float32)
    for c in range(NCORES):
        b, mh, nh = c // 4, (c % 4) // 2, c % 2
        out[b, mh * MSH : (mh + 1) * MSH, nh * NSH : (nh + 1) * NSH] = res.results[
            c
        ]["out"]
    return out, res


def kernel(x, y, x_clip, y_clip):
    out, _ = run_sharded(x, y, x_clip, y_clip, trace=False)
    return out
